# revision 1
# baseline (speedup 1.0000x reference)
"""LoRA Multihead Attention on 8 TRN2 NeuronCores.

Sharding: tensor-parallel over heads. Core c owns heads {2c, 2c+1}
(= channel slice [128c, 128c+128)). Each core:
  1. computes q,k (feature-major) and v (token-major) projections for its heads,
  2. runs attention S^T = k^T q, P = exp(S^T) (softmax denominator via a
     ones-column appended to v, normalization deferred past the P@V matmul),
  3. AllGathers the normalized per-head outputs (channel-sharded, bf16),
  4. computes its 128-column slice of out_proj + LoRA on the gathered output.
Host reassembles the 8 column slices and restores (L, N, E) layout.

All matmuls bf16 with fp32 PSUM accumulation; softmax statistics in fp32.
"""

import sys

sys.path.insert(0, "/opt/trn_rl_repo")

import numpy as np
import ml_dtypes

import concourse.bass as bass  # noqa: F401  (import keeps bass registered)
import concourse.tile as tile
from concourse import bacc, mybir
from concourse.bass_utils import run_bass_kernel_spmd

BF = ml_dtypes.bfloat16
bf16 = mybir.dt.bfloat16
f32 = mybir.dt.float32

L, N, E = 2048, 2, 1024
T = N * L            # 4096 tokens, t = n*L + l
H, D, R = 16, 64, 16
NCORES = 8
HPC = H // NCORES    # heads per core = 2
CS = HPC * D         # channel slice width per core = 128
SCALE = D ** -0.5
LORA_SCALING = 32.0 / 16.0

LB = 512             # l-block (moving free dim)
NT = T // LB         # 8 t-blocks over all tokens
NLB = L // LB        # 4 l-blocks per batch
NMT = L // 128       # 16 m-tiles per batch
NE = E // 128        # 8 contraction tiles

_CACHE = {}


def _build_nc(reps=1, stages=("proj", "attn", "ag", "outproj")):
    nc = bacc.Bacc("TRN2", target_bir_lowering=False, debug=False,
                   enable_asserts=False, num_devices=NCORES)

    qT_d = nc.dram_tensor("qT", [E, T], bf16, kind="ExternalInput")
    wqkt_d = nc.dram_tensor("wqkt", [E, 2 * CS], bf16, kind="ExternalInput")
    wvt_d = nc.dram_tensor("wvt", [E, CS], bf16, kind="ExternalInput")
    bqk_d = nc.dram_tensor("bqk", [2 * CS, 1], f32, kind="ExternalInput")
    woutt_d = nc.dram_tensor("woutt", [E, CS], bf16, kind="ExternalInput")
    at_d = nc.dram_tensor("at", [E, R], bf16, kind="ExternalInput")
    bt_d = nc.dram_tensor("bt", [R, CS], bf16, kind="ExternalInput")
    bout_d = nc.dram_tensor("bout", [CS, 1], f32, kind="ExternalInput")
    outp_d = nc.dram_tensor("outp", [CS, T], f32, kind="ExternalOutput")

    cc_in = [nc.dram_tensor(f"cc_in{n}", [CS, L], bf16) for n in range(N)]
    cc_out = [nc.dram_tensor(f"cc_out{n}", [E, L], bf16, addr_space="Shared")
              for n in range(N)]

    with tile.TileContext(nc) as tc:
        with (
            tc.tile_pool(name="const", bufs=1) as cp,
            tc.tile_pool(name="qt", bufs=1) as qtp,
            tc.tile_pool(name="qks", bufs=1) as qksp,
            tc.tile_pool(name="vp", bufs=1) as vp,
            tc.tile_pool(name="pp", bufs=8) as pp,
            tc.tile_pool(name="osb", bufs=1) as osbp,
            tc.tile_pool(name="ot", bufs=16) as otp,
            tc.tile_pool(name="small", bufs=2) as smp,
            tc.tile_pool(name="ob", bufs=3) as obp,
            tc.tile_pool(name="ps_s", bufs=3, space="PSUM") as ps_s,
            tc.tile_pool(name="ps_acc", bufs=3, space="PSUM") as ps_acc,
            tc.tile_pool(name="ps_m", bufs=2, space="PSUM") as ps_m,
        ):
            # ---- load constants & qT ----
            wqkt = [cp.tile([128, 2 * CS], bf16, tag=f"wqkt{e}", name=f"wqkt{e}") for e in range(NE)]
            wvt = [cp.tile([128, CS], bf16, tag=f"wvt{e}", name=f"wvt{e}") for e in range(NE)]
            woutt = [cp.tile([128, CS], bf16, tag=f"woutt{e}", name=f"woutt{e}") for e in range(NE)]
            at = [cp.tile([128, R], bf16, tag=f"at{e}", name=f"at{e}") for e in range(NE)]
            bt = cp.tile([R, CS], bf16, tag="bt", name="bt")
            bqk = [cp.tile([128, 1], f32, tag=f"bqk{ch}", name=f"bqk{ch}") for ch in range(2)]
            bout = cp.tile([CS, 1], f32, tag="bout", name="bout")
            qt = [qtp.tile([128, T], bf16, tag=f"qt{e}", name=f"qt{e}") for e in range(NE)]
            for e in range(NE):
                sl = slice(e * 128, (e + 1) * 128)
                nc.sync.dma_start(qt[e][:], qT_d.ap()[sl, :])
                nc.sync.dma_start(wqkt[e][:], wqkt_d.ap()[sl, :])
                nc.sync.dma_start(wvt[e][:], wvt_d.ap()[sl, :])
                nc.sync.dma_start(woutt[e][:], woutt_d.ap()[sl, :])
                nc.sync.dma_start(at[e][:], at_d.ap()[sl, :])
            nc.sync.dma_start(bt[:], bt_d.ap())
            nc.sync.dma_start(bqk[0][:], bqk_d.ap()[0:CS, :])
            nc.sync.dma_start(bqk[1][:], bqk_d.ap()[CS:2 * CS, :])
            nc.sync.dma_start(bout[:], bout_d.ap())

            for _rep in range(reps):
              # ---- q,k projection: qks[ch] = (W_{q|k,slice} @ query^T) + bias, bf16
              qks = [qksp.tile([128, T], bf16, tag=f"qks{ch}", name=f"qks{ch}") for ch in range(2)]
              for ch in range(2):
                  for tb in range(NT):
                      pm = ps_m.tile([128, LB], f32, tag="m", name="pm")
                      cs = slice(tb * LB, (tb + 1) * LB)
                      for e in range(NE):
                          nc.tensor.matmul(pm[:], wqkt[e][:, ch * CS:(ch + 1) * CS],
                                           qt[e][:, cs], start=(e == 0), stop=(e == NE - 1))
                      nc.vector.tensor_scalar_add(qks[ch][:, cs], pm[:], bqk[ch][:])

              # ---- v projection, token-major with ones column: v_all[n][h] (128, 16*65)
              v_all = [[vp.tile([128, NMT * (D + 1)], bf16, tag=f"v{n}{h}", name=f"v{n}{h}")
                        for h in range(2)] for n in range(N)]
              for n in range(N):
                  for h in range(2):
                      # ones columns at 64::65 via one strided memset
                      nc.vector.memset(v_all[n][h][:, D::D + 1], 1.0)
              for mt in range(T // 128):
                  pm = ps_m.tile([128, CS], f32, tag="m", name="pmv")
                  cs = slice(mt * 128, (mt + 1) * 128)
                  for e in range(NE):
                      nc.tensor.matmul(pm[:], qt[e][:, cs], wvt[e][:],
                                       start=(e == 0), stop=(e == NE - 1))
                  n, mti = mt // NMT, mt % NMT
                  for h in range(2):
                      nc.vector.tensor_copy(
                          v_all[n][h][:, mti * (D + 1):mti * (D + 1) + D],
                          pm[:, h * D:(h + 1) * D])

              # ---- attention (heads paired for PE row-group concurrency) ----
              # per-batch output tiles so the n=0 AllGather can launch (and
              # overlap) while n=1 attention is still computing
              osb = [osbp.tile([CS, L], bf16, tag=f"osb{n}", name=f"osb{n}")
                     for n in range(N)]
              for n in range(N) if "attn" in stages else []:
                  base = n * L
                  for lb in range(NLB):
                      ls = slice(base + lb * LB, base + (lb + 1) * LB)
                      lsl = slice(lb * LB, (lb + 1) * LB)
                      o_ps = [ps_acc.tile([D + 1, LB], f32, tag="acc", name="ops") for _ in range(2)]
                      for mt in range(NMT):
                          ms = slice(base + mt * 128, base + (mt + 1) * 128)
                          p_t = []
                          for h in range(2):
                              d0 = h * D
                              s_ps = ps_s.tile([128, LB], f32, tag="s")
                              nc.tensor.matmul(s_ps[:], qks[1][d0:d0 + D, ms],
                                               qks[0][d0:d0 + D, ls],
                                               start=True, stop=True)
                              pt = pp.tile([128, LB], bf16, tag="p", name="pt")
                              nc.scalar.activation(pt[:], s_ps[:],
                                                   mybir.ActivationFunctionType.Exp)
                              p_t.append(pt)
                          for h in range(2):
                              vs = slice(mt * (D + 1), mt * (D + 1) + D + 1)
                              nc.tensor.matmul(o_ps[h][:], v_all[n][h][:, vs], p_t[h][:],
                                               start=(mt == 0), stop=(mt == NMT - 1))
                      for h in range(2):
                          rs = smp.tile([1, LB], f32, tag="rs", name="rs")
                          nc.vector.reciprocal(rs[:], o_ps[h][D:D + 1, :])
                          rr = smp.tile([D, LB], f32, tag="rr", name="rr")
                          nc.gpsimd.partition_broadcast(rr[:], rs[:])
                          nc.vector.tensor_mul(osb[n][h * D:(h + 1) * D, lsl],
                                               o_ps[h][0:D, :], rr[:])
                  # ---- AllGather this batch's channel-sharded output now, so
                  # the collective + out_proj DMAs overlap the next batch ----
                  if "ag" in stages:
                      nc.gpsimd.dma_start(cc_in[n].ap(), osb[n][:])
                      nc.gpsimd.collective_compute(
                          "AllGather", mybir.AluOpType.bypass,
                          ins=[cc_in[n].ap()], outs=[cc_out[n].ap()],
                          replica_groups=[list(range(NCORES))],
                      )

              # ---- out_proj + LoRA on the j-column slice of this core ----
              if "outproj" not in stages:
                  if "ag" in stages:
                      gread = otp.tile([128, L], bf16, tag="gread", name="gread", bufs=1)
                      nc.sync.dma_start(gread[:], cc_out[0].ap()[0:128, :])
                      srcs = lambda tb: gread[:, (tb % NLB) * LB:(tb % NLB + 1) * LB]
                  elif "attn" in stages:
                      srcs = lambda tb: osb[tb // NLB][:, (tb % NLB) * LB:(tb % NLB + 1) * LB]
                  else:
                      srcs = lambda tb: qks[0][:, tb * LB:(tb + 1) * LB]
                  for tb in range(NT):
                      cs = slice(tb * LB, (tb + 1) * LB)
                      ob = obp.tile([CS, LB], f32, tag="ob", name="obt2")
                      nc.vector.tensor_copy(ob[:], srcs(tb))
                      nc.sync.dma_start(outp_d.ap()[:, cs], ob[:])
              for tb in range(NT) if "outproj" in stages else []:
                  cs = slice(tb * LB, (tb + 1) * LB)
                  nh, csl = tb // NLB, slice((tb % NLB) * LB, (tb % NLB + 1) * LB)
                  ot = []
                  for e in range(NE):
                      t_ = otp.tile([128, LB], bf16, tag="ot", name="ott")
                      nc.sync.dma_start(t_[:], cc_out[nh].ap()[e * 128:(e + 1) * 128, csl])
                      ot.append(t_)
                  rt_ps = ps_m.tile([R, LB], f32, tag="m", name="rtps")
                  for e in range(NE):
                      nc.tensor.matmul(rt_ps[:], at[e][:], ot[e][:],
                                       start=(e == 0), stop=(e == NE - 1))
                  rt_sb = smp.tile([R, LB], bf16, tag="rt", name="rtsb")
                  nc.vector.tensor_copy(rt_sb[:], rt_ps[:])
                  f_ps = ps_acc.tile([CS, LB], f32, tag="acc", name="fps")
                  for e in range(NE):
                      nc.tensor.matmul(f_ps[:], woutt[e][:], ot[e][:],
                                       start=(e == 0), stop=False)
                  nc.tensor.matmul(f_ps[:], bt[:], rt_sb[:], start=False, stop=True)
                  ob = obp.tile([CS, LB], f32, tag="ob", name="obt")
                  nc.vector.tensor_scalar_add(ob[:], f_ps[:], bout[:])
                  nc.sync.dma_start(outp_d.ap()[:, cs], ob[:])

    nc.compile()
    return nc


def _host_prep(inputs):
    q = np.asarray(inputs["query"], np.float32)
    W = np.asarray(inputs["in_proj_weight"], np.float32)
    b = np.asarray(inputs["in_proj_bias"], np.float32)
    Wout = np.asarray(inputs["out_proj_weight"], np.float32)
    bout = np.asarray(inputs["out_proj_bias"], np.float32)
    A = np.asarray(inputs["lora_A"], np.float32)
    B = np.asarray(inputs["lora_B"], np.float32)

    qT = np.ascontiguousarray(q.transpose(2, 1, 0).reshape(E, T)).astype(BF)
    bv = b[2 * E:3 * E]
    bout_eff = bout + Wout @ bv + LORA_SCALING * (B @ (A @ bv))
    AT = np.ascontiguousarray(A.T).astype(BF)
    BTs = np.ascontiguousarray((B * LORA_SCALING).T)  # (R, E) f32

    in_maps = []
    for c in range(NCORES):
        hs = slice(CS * c, CS * (c + 1))
        wq = W[hs, :] * SCALE
        wk = W[E + CS * c:E + CS * (c + 1), :]
        wv = W[2 * E + CS * c:2 * E + CS * (c + 1), :]
        wqkt = np.ascontiguousarray(np.concatenate([wq.T, wk.T], axis=1)).astype(BF)
        wvt = np.ascontiguousarray(wv.T).astype(BF)
        bqk = np.concatenate([b[hs] * SCALE, b[E + CS * c:E + CS * (c + 1)]])
        in_maps.append({
            "qT": qT,
            "wqkt": wqkt,
            "wvt": wvt,
            "bqk": np.ascontiguousarray(bqk[:, None], np.float32),
            "woutt": np.ascontiguousarray(Wout[hs, :].T).astype(BF),
            "at": AT,
            "bt": np.ascontiguousarray(BTs[:, hs]).astype(BF),
            "bout": np.ascontiguousarray(bout_eff[hs][:, None], np.float32),
        })
    return in_maps


def _run(inputs, trace=False):
    if "nc" not in _CACHE:
        _CACHE["nc"] = _build_nc()
    nc = _CACHE["nc"]
    in_maps = _host_prep(inputs)
    res = run_bass_kernel_spmd(nc, in_maps, core_ids=list(range(NCORES)),
                               trace=trace)
    full = np.empty((E, T), np.float32)
    for c in range(NCORES):
        full[CS * c:CS * (c + 1)] = res.results[c]["outp"]
    out = np.ascontiguousarray(full.reshape(E, N, L).transpose(2, 1, 0))
    return out, res


def kernel(**inputs):
    out, _ = _run(inputs, trace=False)
    return out

